# revision 81
# baseline (speedup 1.0000x reference)
"""CARAFE content-aware upsampling (S=2, K=5) as a Trainium2 Bass/Tile kernel.

v4: restructured reassembly — per (chunk, half) the exp/softmax tile is
PE-transposed once into position-major layout [128=(hp,tw,wp), 104], the
softmax normalization happens post-transpose as a tiny broadcast multiply,
and one GPSIMD local_scatter per half builds the 4-tile M^T [128, 4pq*120]
directly (no pq-replication copies, half the scatter traffic of v3).
Conv taps are packed in pairs via a row-shifted duplicate of xc living in
partitions 64-127 (6 matmuls per chunk instead of 9). Output matmuls write
bf16 PSUM and stage through one [128, 1024] copy per half before DMA.

Sharding: 8 cores = 2 batches x 4 row-quarters (16 low-res rows each).
Per-core pipeline:
  1. content encoder 1x1 conv (PE, bf16): xc -> dupA[0:64]; DVE copies
     build dupA[64:128] = xc shifted one grid row (+68 cols).
  2. per 4-row chunk c (4 chunks): kernel predictor 3x3 conv as 3 paired
     matmuls (contraction 128: taps (0,j)+(1,j)) + 3 single matmuls
     (taps (2,j)); exp(+bias) ACT -> es[0:100]; Z = selT.T @ es (PE);
     reciprocal -> es[100:104] (DVE).
  3. per half u=(c,thl) (8 units):
     a. PE transpose es[:, 128*thl:...] -> esT [128=(hp,tw,wp), 104]
     b. DVE copy -> SBUF; DVE broadcast mul normalizes cols 0:100
     c. GPSIMD local_scatter -> mt4[128, 480=(pq,p)]
     d. 4x PE transpose (per pq) -> m_ps [120, 512]; DVE reorder copy
        -> M_sb [120, (tw,pq,hp,wp)]
     e. 4x PE matmul (per tile): o_ps[128, 1024] bf16 = M.T @ patches
     f. ACT/DVE copy -> ost bf16; DMA out per half.
Host prep (untimed): pad/slice x, bf16 convert, patch tiles, static tables.
"""

import os

os.environ.setdefault("MYCRO_LOCAL_CACHE", "1")

import numpy as np

import ml_dtypes
import concourse.bacc as bacc
import concourse.mybir as mybir
import concourse.tile as tile
from concourse.bass_utils import run_bass_kernel_spmd

F32 = mybir.dt.float32
BF16 = mybir.dt.bfloat16
I16 = mybir.dt.int16
U8 = mybir.dt.uint8
AF = mybir.ActivationFunctionType

B, C, H, W = 2, 256, 64, 64
S, K, COMP = 2, 5, 64
KP = 100          # S*S*K*K
K2 = 25
NCORES = 8
ROWS = 16         # low-res rows per core
RP, WPAD = 20, 68  # padded slice rows/cols
NTH, NTW = 8, 4   # tile grid: 8 x 4 tiles of 2x16 positions
NT = NTH * NTW    # 32 tiles
TPH, TPW = 2, 16  # tile position grid
POS = TPH * TPW   # 32
PATCH = 120       # (TPW+4) * (TPH+4) = 20*6, index = ww*6 + hh
GRID = RP * WPAD  # 1360
NCH = 4           # conv chunks (4 conv rows each)
CW = 256          # conv cols per chunk
NU = 2 * NCH      # units: (chunk, thl)
MT4W = 4 * PATCH  # 480
NWU = int(os.environ.get("K_NWU", "4"))
B_C = int(os.environ.get("K_B_C", "2"))
B_A = int(os.environ.get("K_B_A", "1"))
B_M = int(os.environ.get("K_B_M", "2"))
B_O = int(os.environ.get("K_B_O", "3"))
B_MP = int(os.environ.get("K_B_MP", "3"))
ENC_POOL = int(os.environ.get("K_ENC_POOL", "4"))  # slices >= this on Pool
ENC_DVE = int(os.environ.get("K_ENC_DVE", "0"))    # slices < this on DVE
DIV_POOL = int(os.environ.get("K_DIV_POOL", "0"))  # softmax divide on Pool
E_TW = int(os.environ.get("K_E_TW", "1"))          # split E-copy by tile pair

# x DMA column splits of GRID / encoder compute slices
XSPLITS = [(0, 256), (256, 512), (512, 1024), (1024, GRID)]
ESLICES = XSPLITS

# blob0 layout (bytes per partition): needed at encoder start
OB_WENC = 0                       # [128,2,64] bf16 -> 256B
OB_BENC = OB_WENC + 256           # [64,1] f32      -> 4B
NB0 = OB_BENC + 4
# blob1 layout: needed from conv/reassembly
OB_IDENT = 0                      # [128,128] bf16  -> 256B
OB_IDX = OB_IDENT + 256           # [128,100] i16   -> 200B
OB_WPK = OB_IDX + 200             # [128,3,100] bf16 -> 600B (tap pairs)
OB_WS = OB_WPK + 600              # [64,3,100] bf16  -> 600B (taps (2,j))
OB_IHAT = OB_WS + 600             # [100,104] bf16  -> 208B (I | pq-sum)
OB_BKP = OB_IHAT + 208            # [100,1] f32     -> 4B
NB1 = OB_BKP + 4


def _static_tables():
    # scatter: partition m = hp*64 + tw*16 + wp; src col j = 4*k2 + pq
    # -> dst col pq*120 + (wp+b)*6 + (hp+a)
    idx = np.empty((128, KP), dtype=np.int16)
    for m in range(128):
        hp, wp = m // 64, m % 16
        for j in range(KP):
            k2, pq = j // 4, j % 4
            a, b = k2 // K, k2 % K
            idx[m, j] = pq * PATCH + (wp + b) * 6 + (hp + a)
    ident = np.eye(128, dtype=np.float32).astype(ml_dtypes.bfloat16)
    # A-matmul rhs: transpose identity plus per-pq channel-sum columns
    ihat = np.zeros((KP, 104), dtype=ml_dtypes.bfloat16)
    for ch in range(KP):
        ihat[ch, ch] = 1.0
        ihat[ch, KP + ch % 4] = 1.0
    return idx, ident, ihat


def build_kernel():
    nc = bacc.Bacc("TRN2", target_bir_lowering=False, debug=False)

    xs_c = nc.dram_tensor("xs_c", [C, GRID], BF16, kind="ExternalInput").ap()
    patches_d = nc.dram_tensor(
        "patches", [PATCH, NT * C], BF16, kind="ExternalInput"
    ).ap()
    blob0_d = nc.dram_tensor("blob0", [128, NB0], U8, kind="ExternalInput").ap()
    blob1_d = nc.dram_tensor("blob1", [128, NB1], U8, kind="ExternalInput").ap()
    out_d = nc.dram_tensor("out", [128, NT * C], BF16, kind="ExternalOutput").ap()

    with tile.TileContext(nc) as tc:
        _build(tc, nc, xs_c, patches_d, blob0_d, blob1_d, out_d)
    nc.compile()
    return nc


def _build(tc, nc, xs_c, patches_d, blob0_d, blob1_d, out_d):
    with (
        tc.tile_pool(name="const", bufs=1) as cpool,
        tc.tile_pool(name="work", bufs=1) as wpool,
        tc.tile_pool(name="cp", bufs=3) as cp,
        tc.tile_pool(name="ap", bufs=3) as ap,
        tc.tile_pool(name="mp", bufs=B_MP) as mp,
        tc.tile_pool(name="opool", bufs=3) as opool,
        tc.tile_pool(name="ps_c", bufs=B_C, space="PSUM") as ps_c,
        tc.tile_pool(name="ps_a", bufs=B_A, space="PSUM") as ps_a,
        tc.tile_pool(name="ps_m", bufs=B_M, space="PSUM") as ps_m,
        tc.tile_pool(name="ps_o", bufs=B_O, space="PSUM") as ps_o,
    ):
        # ---- warmup source (no DMA dependency)
        wu_sb = cpool.tile([128, 512], BF16, tag="wu")
        nc.gpsimd.memset(wu_sb[:], 0.0)

        # ---- DMAs: blob0 on ACT queue; blob1 + x slices + patches on SP
        blob0_sb = cpool.tile([128, NB0], U8, tag="blob0")
        nc.scalar.dma_start(blob0_sb[:], blob0_d)

        x_sb = cpool.tile([128, 2, GRID], BF16, tag="x")
        xg = xs_c.rearrange("(blk p) f -> p blk f", p=128)
        for lo, hi in XSPLITS[0:2]:
            nc.sync.dma_start(x_sb[:, :, lo:hi], xg[:, :, lo:hi])
        blob1_sb = cpool.tile([128, NB1], U8, tag="blob1")
        nc.sync.dma_start(blob1_sb[:], blob1_d)
        for lo, hi in XSPLITS[2:]:
            nc.sync.dma_start(x_sb[:, :, lo:hi], xg[:, :, lo:hi])
        pat_sb = cpool.tile([PATCH, NT * C], BF16, tag="pat")
        NQ = NT * C // 4
        for qi in range(4):
            nc.sync.dma_start(
                pat_sb[:, qi * NQ:(qi + 1) * NQ],
                patches_d[:, qi * NQ:(qi + 1) * NQ])

        wenc_sb = blob0_sb[:, OB_WENC:OB_WENC + 256].bitcast(BF16).rearrange(
            "p (blk m) -> p blk m", blk=2)
        benc_sb = blob0_sb[0:COMP, OB_BENC:OB_BENC + 4].bitcast(F32)
        ident_sb = blob1_sb[:, OB_IDENT:OB_IDENT + 256].bitcast(BF16)
        idx_sb = blob1_sb[:, OB_IDX:OB_IDX + 200].bitcast(I16)
        wpk_sb = blob1_sb[:, OB_WPK:OB_WPK + 600].bitcast(BF16).rearrange(
            "p (j m) -> p j m", j=3)
        ws_sb = blob1_sb[0:COMP, OB_WS:OB_WS + 600].bitcast(BF16).rearrange(
            "p (j m) -> p j m", j=3)
        ihat_sb = blob1_sb[0:KP, OB_IHAT:OB_IHAT + 208].bitcast(BF16)
        bkp_sb = blob1_sb[0:KP, OB_BKP:OB_BKP + 4].bitcast(F32)



        # ---- PE warmup: accumulating junk matmuls keep the p-state ramp
        # alive while DMAs land (emitted in spurts between real stages)
        def warm(n):
            wu_ps = ps_a.tile([128, KP], F32, tag="a", name="wu_ps")[:]
            for i in range(n):
                nc.tensor.matmul(wu_ps, wu_sb[:, 0:128], wu_sb[:, 0:KP],
                                 start=(i == 0), stop=(i == n - 1))

        # ---- phase 1: encoder 1x1 conv -> dupA[0:64] = xc, then
        #      dupA[64:128, g] = xc[:, g+68] (one grid row down)
        dupA = wpool.tile([128, GRID], BF16, tag="xc")

        def enc_slice(si):
            lo, hi = ESLICES[si]
            enc_ps = ps_o.tile([COMP, 512], F32, tag="o", name="enc_ps")
            for blk in range(2):
                nc.tensor.matmul(
                    enc_ps[:, 0:hi - lo],
                    wenc_sb[:, blk, :],
                    x_sb[:, blk, lo:hi],
                    start=(blk == 0), stop=(blk == 1),
                )
            if si < ENC_DVE:
                # DVE bias-add: the following dup copy is on the same queue,
                # removing a cross-engine hop from the conv(0) chain
                with nc.allow_low_precision(reason="xc in bf16 as in v3"):
                    nc.vector.tensor_tensor(
                        dupA[0:COMP, lo:hi], enc_ps[:, 0:hi - lo],
                        benc_sb.broadcast_to([COMP, hi - lo]),
                        op=mybir.AluOpType.add)
            elif si < ENC_POOL:
                nc.scalar.activation(
                    dupA[0:COMP, lo:hi], enc_ps[:, 0:hi - lo], AF.Identity,
                    bias=benc_sb)
            else:
                # keep the ACT queue clear for exp(c0): late slices' bias
                # adds run on the (idle-until-scatter) Pool engine
                with nc.allow_low_precision(reason="xc in bf16 as in v3"):
                    nc.gpsimd.tensor_tensor(
                        dupA[0:COMP, lo:hi], enc_ps[:, 0:hi - lo],
                        benc_sb.broadcast_to([COMP, hi - lo]),
                        op=mybir.AluOpType.add)
            dlo, dhi = max(lo - 68, 0), hi - 68
            nc.vector.tensor_copy(
                dupA[COMP:128, dlo:dhi], dupA[0:COMP, dlo + 68:hi])

        dupA_g = dupA[:].rearrange("p (r w) -> p r w", r=RP)

        # ---- per-chunk conv + softmax front, per-half reassembly
        es_t = [None] * NCH
        mt4_t = [None] * NU
        msb_t = [None] * NU
        ops_t = [None] * NU

        def conv(c):
            kp_ps = ps_c.tile([KP, CW], F32, tag="c", name="kp_ps")
            r0 = 1 + 4 * c
            for j in range(3):
                nc.tensor.matmul(
                    kp_ps[:],
                    wpk_sb[:, j, :],
                    dupA_g[:, r0:r0 + 4, 1 + j:65 + j],
                    start=(j == 0), stop=False,
                )
            for j in range(3):
                nc.tensor.matmul(
                    kp_ps[:],
                    ws_sb[:, j, :],
                    dupA_g[0:COMP, r0 + 2:r0 + 6, 1 + j:65 + j],
                    start=False, stop=(j == 2),
                )
            es_sb = cp.tile([KP, CW], BF16, tag="es")
            nc.scalar.activation(es_sb[:], kp_ps[:], AF.Exp, bias=bkp_sb)
            es_t[c] = es_sb

        def stage_a(u):
            # one matmul vs [I|pq-sum] -> pt_ps [128=(hp,tw,wp), 100 esT
            # cols + 4 Z cols]; fused divide+downconvert (DVE) -> prob_sb;
            # scatter into 4-tile M^T
            c, thl = u // 2, u % 2
            es_sb = es_t[c]
            pt_ps = ps_a.tile([128, 104], F32, tag="a", name="pt_ps")
            nc.tensor.matmul(
                pt_ps[:], es_sb[:, 128 * thl:128 * (thl + 1)], ihat_sb,
                start=True, stop=True)
            prob_sb = ap.tile([128, KP], BF16, tag="prob")
            prob = prob_sb[:].rearrange("p (k q) -> p k q", k=K2)
            src = pt_ps[:, 0:KP].rearrange("p (k q) -> p k q", k=K2)
            # DVE has no divide and may read only one non-scalar input from
            # PSUM: reciprocal the 4 Z columns into SBUF, then multiply
            rz_sb = ap.tile([128, 4], BF16, tag="zt")
            with nc.allow_low_precision(reason="recip feeds bf16 mults"):
                nc.vector.reciprocal(rz_sb[:], pt_ps[:, KP:104])
            zb = rz_sb[:].unsqueeze(1).broadcast_to([128, K2, 4])
            with nc.allow_low_precision(reason="softmax probs in bf16"):
                nc.vector.tensor_tensor(
                    prob, src, zb, op=mybir.AluOpType.mult)
            mt4 = mp.tile([128, MT4W], BF16, tag="mt4")
            nc.gpsimd.local_scatter(
                mt4[:], prob_sb[:], idx_sb[:, 0:KP],
                channels=128, num_elems=MT4W, num_idxs=KP,
            )
            mt4_t[u] = mt4

        def stage_b(u):
            # 4 per-pq transposes -> m_ps [120, (pq,hp,tw,wp)]; reorder copy
            # -> M_sb [120, (tw,pq,hp,wp)]
            mt4 = mt4_t[u]
            m_ps = ps_m.tile([PATCH, 512], BF16, tag="m", name="m_ps")
            for pq in range(4):
                nc.tensor.transpose(
                    m_ps[:, 128 * pq:128 * (pq + 1)],
                    mt4[:, PATCH * pq:PATCH * (pq + 1)], ident_sb)
            m_sb = mp.tile([PATCH, 512], BF16, tag="msb")
            src = m_ps[:].rearrange(
                "p (pq hp tw wp) -> p pq hp tw wp", pq=4, hp=2, tw=4)
            dst = m_sb[:].rearrange(
                "p (tw pq hp wp) -> p pq hp tw wp", tw=4, pq=4, hp=2)
            if E_TW:
                nc.vector.tensor_copy(dst[:, :, :, 0:2], src[:, :, :, 0:2])
                nc.scalar.copy(dst[:, :, :, 2:4], src[:, :, :, 2:4])
            else:
                nc.vector.tensor_copy(dst[:, 0:2], src[:, 0:2])
                nc.scalar.copy(dst[:, 2:4], src[:, 2:4])
            msb_t[u] = m_sb

        def stage_c(u):
            # 4 per-tile output matmuls -> 2x o_ps [128=(pq,hp,wp), 2*C] f32,
            # staged into ost bf16 (ACT first half, DVE second)
            m_sb = msb_t[u]
            t0 = 4 * u
            ost = opool.tile([128, 4 * C], BF16, tag="ost", name="ost_t")
            for half in range(2):
                o_ps = ps_o.tile([128, 2 * C], F32, tag="o", name="o_ps")
                for k in range(2):
                    tw = 2 * half + k
                    nc.tensor.matmul(
                        o_ps[:, C * k:C * (k + 1)],
                        m_sb[:, 128 * tw:128 * (tw + 1)],
                        pat_sb[:, (t0 + tw) * C:(t0 + tw + 1) * C],
                        start=True, stop=True)
                dst = ost[:, 2 * C * half:2 * C * (half + 1)]
                if half == 0:
                    nc.scalar.copy(dst, o_ps[:])
                else:
                    nc.vector.tensor_copy(dst, o_ps[:])
            ops_t[u] = ost

        def stage_d(u):
            ost = ops_t[u]
            g = 4 * u * C
            if u == NU - 1:
                # split the final DMA so the first half ships while the
                # second half's copy completes
                nc.sync.dma_start(out_d[:, g:g + 2 * C], ost[:, 0:2 * C])
                nc.sync.dma_start(
                    out_d[:, g + 2 * C:g + 4 * C], ost[:, 2 * C:4 * C])
            else:
                nc.sync.dma_start(out_d[:, g:g + 4 * C], ost[:])

        # software-pipelined emission: keep PE stream dense; stage_a(u)
        # needs conv(u//2); stage_c needs stage_b needs stage_a. Warmup
        # spurts bridge PE idle gaps during the DMA-bound preamble so the
        # p-state ramp survives into the real work.
        warm(4)
        enc_slice(0)
        enc_slice(1)
        warm(2)
        conv(0)
        enc_slice(2)
        stage_a(0)
        stage_a(1)
        enc_slice(3)
        conv(1)
        stage_b(0)
        conv(2)
        stage_a(2)
        stage_b(1)
        stage_c(0)
        stage_d(0)
        conv(3)
        stage_a(3)
        stage_b(2)
        stage_c(1)
        stage_d(1)
        stage_a(4)
        stage_b(3)
        stage_c(2)
        stage_d(2)
        stage_a(5)
        stage_b(4)
        stage_c(3)
        stage_d(3)
        stage_a(6)
        stage_b(5)
        stage_c(4)
        stage_d(4)
        stage_a(7)
        stage_b(6)
        stage_c(5)
        stage_d(5)
        stage_b(7)
        stage_c(6)
        stage_d(6)
        stage_c(7)
        stage_d(7)


def host_prep(x, w_enc, b_enc, w_kp, b_kp):
    """Build per-core input maps (pure relayout, untimed)."""
    idx, ident, ihat = _static_tables()
    xpad = np.pad(x, ((0, 0), (0, 0), (2, 2), (2, 2)))  # [B, C, 68, 68]
    w_encT = np.ascontiguousarray(w_enc.T)              # [256, 64]
    w_kp9 = np.ascontiguousarray(
        np.transpose(w_kp, (1, 2, 3, 0)).reshape(COMP, 9 * KP)
    ).astype(ml_dtypes.bfloat16)

    blob0 = np.zeros((128, NB0), np.uint8)
    wenc_b = w_encT.astype(ml_dtypes.bfloat16).reshape(2, 128, COMP)
    wenc_b = np.ascontiguousarray(wenc_b.transpose(1, 0, 2)).reshape(128, 128)
    blob0[:, OB_WENC:OB_WENC + 256] = wenc_b.view(np.uint8)
    blob0[0:COMP, OB_BENC:OB_BENC + 4] = np.ascontiguousarray(
        np.asarray(b_enc, np.float32).reshape(COMP, 1)).view(np.uint8)

    blob1 = np.zeros((128, NB1), np.uint8)
    blob1[:, OB_IDENT:OB_IDENT + 256] = ident.view(np.uint8).reshape(128, 256)
    blob1[:, OB_IDX:OB_IDX + 200] = idx.view(np.uint8).reshape(128, 200)
    # conv tap pairs: rows 0:64 tap (0,j), rows 64:128 tap (1,j)
    wpk = np.zeros((128, 3, KP), dtype=ml_dtypes.bfloat16)
    for j in range(3):
        wpk[0:COMP, j] = w_kp9[:, j * KP:(j + 1) * KP]
        wpk[COMP:128, j] = w_kp9[:, (3 + j) * KP:(4 + j) * KP]
    blob1[:, OB_WPK:OB_WPK + 600] = wpk.reshape(128, 300).view(np.uint8)
    wsing = np.zeros((COMP, 3, KP), dtype=ml_dtypes.bfloat16)
    for j in range(3):
        wsing[:, j] = w_kp9[:, (6 + j) * KP:(7 + j) * KP]
    blob1[0:COMP, OB_WS:OB_WS + 600] = wsing.reshape(COMP, 300).view(np.uint8)
    blob1[0:KP, OB_IHAT:OB_IHAT + 208] = np.ascontiguousarray(ihat).view(
        np.uint8).reshape(KP, 208)
    blob1[0:KP, OB_BKP:OB_BKP + 4] = np.ascontiguousarray(
        np.asarray(b_kp, np.float32).reshape(KP, 1)).view(np.uint8)

    in_maps = []
    for core in range(NCORES):
        b, q = core // 4, core % 4
        sl = xpad[b, :, 16 * q: 16 * q + RP, :]          # [C, 20, 68]
        xs_c = np.ascontiguousarray(sl.reshape(C, GRID)).astype(
            ml_dtypes.bfloat16)
        # patch tiles: [PATCH, NT, C], p = ww*6 + hh
        pat = np.empty((NT, PATCH, C), dtype=ml_dtypes.bfloat16)
        for t in range(NT):
            th, tw = t // NTW, t % NTW
            blk = sl[:, 2 * th: 2 * th + 6, TPW * tw: TPW * tw + 20]  # [C,6,20]
            pat[t] = np.transpose(blk, (2, 1, 0)).reshape(PATCH, C)
        pat = np.ascontiguousarray(np.transpose(pat, (1, 0, 2)))
        in_maps.append({
            "xs_c": xs_c,
            "patches": pat.reshape(PATCH, NT * C),
            "blob0": blob0,
            "blob1": blob1,
        })
    return in_maps


def host_assemble(results):
    """results: list of 8 dicts with 'out' [128, NT*C] -> full [B, C, 128, 128]."""
    out = np.empty((B, C, H * S, W * S), dtype=np.float32)
    for core in range(NCORES):
        b, q = core // 4, core % 4
        # out rows m = pq*32 + hp*16 + wp, cols (t, c)
        a = results[core]["out"].astype(np.float32).reshape(
            2, 2, TPH, TPW, NTH, NTW, C)
        # dims: p, q2, hp, wp, th, tw, c -> [c, th, hp, p, tw, wp, q2]
        o = np.transpose(a, (6, 4, 2, 0, 5, 3, 1)).reshape(C, 32, 128)
        out[b, :, 32 * q: 32 * (q + 1), :] = o
    return out


_NC_CACHE = None


def kernel(x, w_enc, b_enc, w_kp, b_kp):
    global _NC_CACHE
    x = np.asarray(x)
    w_enc = np.asarray(w_enc)
    b_enc = np.asarray(b_enc)
    w_kp = np.asarray(w_kp)
    b_kp = np.asarray(b_kp)
    if _NC_CACHE is None:
        _NC_CACHE = build_kernel()
    nc = _NC_CACHE
    in_maps = host_prep(x, w_enc, b_enc, w_kp, b_kp)
    trace = os.environ.get("CARAFE_TRACE", "0") == "1"
    res = run_bass_kernel_spmd(nc, in_maps, list(range(NCORES)), trace=trace)
    out = host_assemble(res.results)
    if trace:
        kernel.last_exec_time_ns = res.exec_time_ns
        kernel.last_results = res
    return out
